# revision 1
# baseline (speedup 1.0000x reference)
"""3-layer GCN (GCNConv x3) distributed over 8 NeuronCores — v2.

Differences from v1 (kernel.py):
- Self-loops leave the gather stream: per block, one PE matmul accumulates
  hsc = h_local * self_coef into the agg PSUM via an identity rhs (layer 0's
  hsc comes precomputed from the host as `xsc`; later layers build it on DVE
  from the block's hout).
- Tiles pack across block boundaries within a (window, chunk) group: a
  128-edge tile may span 2+ blocks.  dstl encodes chunk-relative ids
  (128*(b-b0)+d); one extra matmul per (tile, covered-block) pair, with the
  pair list unioned across cores so the SPMD program is identical everywhere
  (foreign pairs accumulate zeros).  Gather slot padding drops from ~30% to
  ~4%.
- Eq matrices are built batched: one DVE tensor_tensor per (window, chunk,
  block-slot) over the covered tile range, against an iota4 [P, 512] constant.
"""

import sys

sys.path.insert(0, "/opt/trn_rl_repo")

import numpy as np

from concourse import bass, bacc, mybir, tile
from concourse import bass_utils

P = 128
WIN = 32768  # int16 index window


def preprocess(x, edge_src, edge_dst, edge_weights, n_cores=8, bpc=4):
    N, F = x.shape
    E = edge_src.shape[0]
    C = n_cores

    w64 = edge_weights.astype(np.float64)
    deg = np.bincount(edge_dst, weights=w64, minlength=N) + 1.0
    dinv = 1.0 / np.sqrt(deg)
    norm = (dinv[edge_src] * w64 * dinv[edge_dst]).astype(np.float32)
    self_coef = (dinv * dinv).astype(np.float32)

    indeg = np.bincount(edge_dst, minlength=N)
    rounds = indeg + 1

    # deal nodes by descending degree: rank r -> core r%C, pos r//C
    order = np.argsort(-rounds, kind="stable")
    core_of = np.empty(N, np.int64)
    pos_of = np.empty(N, np.int64)
    r = np.arange(N)
    core_of[order] = r % C
    pos_of[order] = r // C

    npc = N // C
    B = (npc + P - 1) // P
    rows_pc = B * P
    rows_total = C * rows_pc
    pid = core_of * rows_pc + pos_of
    blk_of = pos_of // P
    K = (rows_total + WIN - 1) // WIN

    chunks = [(b, min(b + bpc, B)) for b in range(0, B, bpc)]
    NCH = len(chunks)
    ch_of_blk = np.zeros(B, np.int64)
    for ci, (b0, b1) in enumerate(chunks):
        ch_of_blk[b0:b1] = ci

    e_core = core_of[edge_dst]
    e_blk = blk_of[edge_dst]
    e_ch = ch_of_blk[e_blk]
    e_pid_src = pid[edge_src]
    e_win = e_pid_src // WIN
    e_lidx = (e_pid_src % WIN).astype(np.int32)
    e_d = (pos_of[edge_dst] % P).astype(np.int32)

    # sort by (core, window, chunk, block); edges of a group laid densely
    key = ((e_core * K + e_win) * B + e_blk)  # block implies chunk
    sort_e = np.argsort(key, kind="stable")
    cnt_ckb = np.bincount(key, minlength=C * K * B).reshape(C, K, B)

    # per (c,k,ci) group sizes, tiles per (k,ci) = max over cores
    cnt_ckc = np.zeros((C, K, NCH), np.int64)
    for ci, (b0, b1) in enumerate(chunks):
        cnt_ckc[:, :, ci] = cnt_ckb[:, :, b0:b1].sum(axis=2)
    Tkc = np.maximum.reduce(-(-cnt_ckc // P), axis=0)  # [K, NCH]
    Tkc = np.maximum(Tkc, 1)  # keep >=1 col per group for simplicity
    choff = np.zeros((K, NCH + 1), np.int64)
    choff[:, 1:] = np.cumsum(Tkc, axis=1)
    TOTk = choff[:, -1].copy()
    win_base = np.zeros(K + 1, np.int64)
    win_base[1:] = np.cumsum(TOTk)
    TOT = int(win_base[-1])

    # edge position within its (c,k,ci) group (block-sorted)
    gkey = (e_core * K + e_win) * NCH + e_ch
    gkey_s = gkey[sort_e]
    n_groups = C * K * NCH
    gcnt = np.bincount(gkey_s, minlength=n_groups)
    gfirst = np.zeros(n_groups + 1, np.int64)
    gfirst[1:] = np.cumsum(gcnt)
    jpos = np.arange(E) - gfirst[gkey_s]

    es = sort_e
    tile_rel = jpos // P
    slot = jpos % P
    col_w = choff[e_win[es], e_ch[es]] + tile_rel
    col_g = win_base[e_win[es]] + col_w
    # chunk-relative dst encoding
    b0_arr = np.array([c0 for (c0, _) in chunks])
    enc = (e_blk[es] - b0_arr[e_ch[es]]) * P + e_d[es]
    ecore = e_core[es]

    # pad indices are -1: trailing negatives are skipped by the gather DGE
    # (no descriptor emitted), with num_idxs_reg giving the per-core count
    gidx_flat = [np.full((C, max(1, int(TOTk[k])) * P), -1, np.int16)
                 for k in range(K)]
    coef = np.zeros((C, P, TOT), np.float32)
    dstl = np.full((C, P, TOT), 999.0, np.float32)

    ew = e_win[es]
    for k in range(K):
        m = ew == k
        gidx_flat[k][ecore[m], col_w[m] * P + slot[m]] = e_lidx[es][m].astype(
            np.int16
        )
    coef[ecore, slot, col_g] = norm[es]
    dstl[ecore, slot, col_g] = enc.astype(np.float32)

    # union matmul pair list: per (k,ci): sorted (t, s) pairs present in ANY
    # core, plus per-(b) last-pair bookkeeping
    pairs = {}
    for k in range(K):
        for ci, (b0, b1) in enumerate(chunks):
            pset = set()
            for c in range(C):
                cum = 0
                for s, b in enumerate(range(b0, b1)):
                    nb = int(cnt_ckb[c, k, b])
                    if nb == 0:
                        cum += 0
                        continue
                    t_lo = cum // P
                    t_hi = (cum + nb - 1) // P
                    for t in range(t_lo, t_hi + 1):
                        pset.add((t, s))
                    cum += nb
            pairs[(k, ci)] = sorted(pset)

    # per-core valid-index counts per gather call (call = (k, ci))
    assert (cnt_ckc >= 1).all(), "gather group with zero edges"
    ncnt = cnt_ckc.reshape(C, K * NCH).astype(np.int32)

    # per-window int16 index streams wrapped in 16 partitions, replicated x8
    gidx16 = []
    for k in range(K):
        nidx = gidx_flat[k].shape[1]
        w = gidx_flat[k].reshape(C, nidx // 16, 16).transpose(0, 2, 1)
        gidx16.append(np.tile(w, (1, 8, 1)).astype(np.int16))

    xt = np.zeros((rows_total, F), np.float32)
    xt[pid] = np.asarray(x, np.float32)

    # per-core hsc inputs: xsc[d, b*F+f] = x_local * self_coef (fp16), and
    # scp[d, b] = self_coef for on-device hsc builds in later layers
    xsc = np.zeros((C, P, B * F), np.float16)
    scp = np.zeros((C, P, B), np.float32)
    xl = xt.reshape(C, B, P, F)
    scl = np.zeros((C, rows_pc), np.float32)
    scl[core_of, pos_of] = self_coef
    scl = scl.reshape(C, B, P)
    for c in range(C):
        hs = xl[c] * scl[c][:, :, None]          # [B, P, F]
        xsc[c] = hs.transpose(1, 0, 2).reshape(P, B * F).astype(np.float16)
        scp[c] = scl[c].transpose(1, 0)           # [P, B]

    return dict(
        C=C, N=N, F=F, B=B, K=K, bpc=bpc, chunks=chunks,
        Tkc=Tkc, choff=choff, TOTk=[int(t) for t in TOTk],
        win_base=[int(w) for w in win_base], TOT=TOT, pairs=pairs,
        cnt_ckb=cnt_ckb,
        rows_pc=rows_pc, rows_total=rows_total,
        core_of=core_of, pos_of=pos_of, NCH=NCH,
        xt=xt, gidx16=gidx16, coef=coef, dstl=dstl, xsc=xsc, scp=scp,
        ncnt=ncnt,
    )


def build_nc(meta, skip_collective=False, scratch=16384, n_queues=4,
             eq_mode="batched", pad_skip=True):
    C = meta["C"]; F = meta["F"]; B = meta["B"]; K = meta["K"]
    chunks = meta["chunks"]; Tkc = meta["Tkc"]; choff = meta["choff"]
    TOTk = meta["TOTk"]; win_base = meta["win_base"]; TOT = meta["TOT"]
    pairs = meta["pairs"]
    rows_pc = meta["rows_pc"]; rows_total = meta["rows_total"]
    dt = mybir.dt
    f32 = dt.float32
    f16 = dt.float16

    CHMAX = int(max(Tkc[k][ci] for k in range(K) for ci in range(len(chunks))))
    # widest batched-eq span needed for block-slots s >= 1
    SMAX = 1
    for k in range(K):
        for ci in range(len(chunks)):
            for s in set(s_ for (_, s_) in pairs[(k, ci)] if s_ > 0):
                ts = [t for (t, s_) in pairs[(k, ci)] if s_ == s]
                SMAX = max(SMAX, max(ts) - min(ts) + 1)

    # last (k,ci,t,s) pair per block, for PSUM stop flags
    last_pair = {}
    for k in range(K):
        for ci, (b0, b1) in enumerate(chunks):
            for (t, s) in pairs[(k, ci)]:
                last_pair[b0 + s] = (k, ci, t, s)
    assert all(b in last_pair for b in range(B)), "block with no edge tiles"

    nc = bacc.Bacc("TRN2", target_bir_lowering=False, debug=False, num_devices=C,
                   dynamic_dma_scratch_size=scratch, num_swdge_queues=n_queues)

    xt = nc.dram_tensor("xt", [rows_total, F], f32, kind="ExternalInput").ap()
    gixd = [
        nc.dram_tensor(f"gix{k}", [P, max(1, TOTk[k]) * 8], dt.int16,
                       kind="ExternalInput").ap()
        for k in range(K)
    ]
    coef_d = nc.dram_tensor("coef", [P, TOT], f32, kind="ExternalInput").ap()
    dstl_d = nc.dram_tensor("dstl", [P, TOT], f32, kind="ExternalInput").ap()
    SBLK = meta["bpc"]
    iota4_d = nc.dram_tensor("iota4", [P, SBLK * P], f16,
                            kind="ExternalInput").ap()
    id128_d = nc.dram_tensor("id128", [P, P], f16, kind="ExternalInput").ap()
    ident_d = nc.dram_tensor("ident", [F, F], f32, kind="ExternalInput").ap()
    xsc_d = nc.dram_tensor("xsc", [P, B * F], f16, kind="ExternalInput").ap()
    scp_d = nc.dram_tensor("scp", [P, B], f32, kind="ExternalInput").ap()
    NCALL = meta["ncnt"].shape[1]
    ncnt_d = nc.dram_tensor("ncnt", [1, NCALL], dt.int32, kind="ExternalInput").ap()
    w_d = [
        nc.dram_tensor(f"w{i}", [F, F if i < 2 else 1], f32, kind="ExternalInput").ap()
        for i in range(3)
    ]
    b_d = [
        nc.dram_tensor(f"b{i}", [F, 1], f32, kind="ExternalInput").ap()
        for i in range(2)
    ]
    y_d = nc.dram_tensor("y", [1, rows_pc], f32, kind="ExternalOutput").ap()

    hloc = [nc.dram_tensor(f"hloc{i}", [rows_pc, F], f32) for i in range(2)]
    htab = [
        nc.dram_tensor(f"htab{i}", [rows_total, F], f32, addr_space="Shared")
        for i in range(2)
    ]

    nbuf = 3 if CHMAX <= 32 else 2
    with tile.TileContext(nc) as tc:
        with (
            tc.tile_pool(name="const", bufs=1) as cpool,
            tc.tile_pool(name="gather", bufs=nbuf) as gpool,
            tc.tile_pool(name="msgs", bufs=nbuf) as mpool,
            tc.tile_pool(name="eqp", bufs=nbuf) as epool,
            tc.tile_pool(name="eqs", bufs=6) as espool,
            tc.tile_pool(name="aggs", bufs=3) as apool,
            tc.tile_pool(name="hout", bufs=3) as hpool,
            tc.tile_pool(name="psum_agg", bufs=4, space="PSUM") as ps_agg,
            tc.tile_pool(name="psum_dense", bufs=2, space="PSUM") as ps_dense,
            tc.tile_pool(name="psum_tr", bufs=2, space="PSUM") as ps_tr,
        ):
            gix_sb = [
                cpool.tile([P, max(1, TOTk[k]) * 8], dt.int16, tag=f"gix{k}",
                           name=f"gix{k}sb")
                for k in range(K)
            ]
            coef_sb = cpool.tile([P, TOT], f32, tag="coef")
            dstl_sb = cpool.tile([P, TOT], f32, tag="dstl")
            iota4_sb = cpool.tile([P, SBLK * P], f16, tag="iota4")
            id128_sb = cpool.tile([P, P], f16, tag="id128")
            ident_sb = cpool.tile([F, F], f32, tag="ident")
            xsc_sb = cpool.tile([P, B * F], f16, tag="xsc")
            scp_sb = cpool.tile([P, B], f32, tag="scp")
            ncnt_sb = cpool.tile([1, NCALL], dt.int32, tag="ncnt")
            hsc_sb = [cpool.tile([P, B * F], f16, tag=f"hsc{i}",
                                 name=f"hsc{i}sb") for i in range(2)]
            w_sb = [cpool.tile([F, F if i < 2 else 1], f32, tag=f"w{i}",
                               name=f"w{i}sb") for i in range(3)]
            b_sb = [cpool.tile([F, 1], f32, tag=f"b{i}", name=f"b{i}sb")
                    for i in range(2)]
            y_sb = cpool.tile([1, rows_pc], f32, tag="ysb")

            for k in range(K):
                nc.sync.dma_start(out=gix_sb[k][:, :], in_=gixd[k][:, :])
            nc.sync.dma_start(out=coef_sb[:, :], in_=coef_d[:, :])
            nc.sync.dma_start(out=dstl_sb[:, :], in_=dstl_d[:, :])
            nc.sync.dma_start(out=iota4_sb[:, :], in_=iota4_d[:, :])
            nc.sync.dma_start(out=id128_sb[:, :], in_=id128_d[:, :])
            nc.sync.dma_start(out=ident_sb[:, :], in_=ident_d[:, :])
            nc.sync.dma_start(out=xsc_sb[:, :], in_=xsc_d[:, :])
            nc.sync.dma_start(out=scp_sb[:, :], in_=scp_d[:, :])
            nc.sync.dma_start(out=ncnt_sb[:, :], in_=ncnt_d[:, :])
            gcnt_reg = (nc.alloc_register(mybir.EngineType.Pool, "gcnt")
                        if pad_skip else None)
            for i in range(3):
                nc.sync.dma_start(out=w_sb[i][:, :], in_=w_d[i][:, :])
            for i in range(2):
                nc.sync.dma_start(out=b_sb[i][:, :], in_=b_d[i][:, :])

            call_no = 0
            for L in range(3):
                table = [xt, htab[0][:, :], htab[1][:, :]][L]
                hsc_cur = [xsc_sb, hsc_sb[0], hsc_sb[1]][L]
                for ci, (b0, b1) in enumerate(chunks):
                    aggs_ps = {}
                    for b in range(b0, b1):
                        aggs_ps[b] = ps_agg.tile([F, P], f32, tag="agg",
                                                 name=f"agg{L}_{b}")
                        # self-loop: agg += hsc_b^T via identity rhs
                        nc.tensor.matmul(
                            aggs_ps[b][:, :],
                            lhsT=hsc_cur[:, b * F:(b + 1) * F],
                            rhs=id128_sb[:, :],
                            start=True,
                            stop=False,
                        )
                    for k in range(K):
                        cw0 = int(choff[k][ci]); cw1 = int(choff[k][ci + 1])
                        cols = cw1 - cw0
                        gc0 = win_base[k] + cw0
                        gc1 = win_base[k] + cw1
                        g = gpool.tile([P, CHMAX * F], f32, tag="g")
                        ci_call = k * meta["NCH"] + ci
                        if pad_skip:
                            nc.gpsimd.reg_load(
                                gcnt_reg, ncnt_sb[0:1, ci_call:ci_call + 1])
                            nreg = gcnt_reg
                        else:
                            nreg = cols * P
                        nc.gpsimd.dma_gather(
                            out_ap=g[:, : cols * F].rearrange(
                                "p (t f) -> p t f", f=F),
                            in_ap=table[k * WIN: min((k + 1) * WIN, rows_total), :],
                            idxs_ap=gix_sb[k][:, cw0 * 8: cw1 * 8],
                            num_idxs=cols * P,
                            num_idxs_reg=nreg,
                            elem_size=F,
                            single_packet=False,
                            queue_num=call_no % n_queues,
                        )
                        call_no += 1
                        m = mpool.tile([P, CHMAX * F], f16, tag="m")
                        nc.vector.tensor_tensor(
                            out=m[:, : cols * F].rearrange("p (t f) -> p t f", f=F),
                            in0=g[:, : cols * F].rearrange("p (t f) -> p t f", f=F),
                            in1=coef_sb[:, gc0:gc1].to_broadcast([P, cols, F]),
                            op=mybir.AluOpType.mult,
                        )
                        plist = pairs[(k, ci)]
                        eq_of = {}
                        if eq_mode == "batched":
                            # batched eq per block-slot s over its tile range
                            for s in sorted(set(s_ for (_, s_) in plist)):
                                ts = [t for (t, s_) in plist if s_ == s]
                                tA, tB = min(ts), max(ts)
                                span = tB - tA + 1
                                pool = epool if s == 0 else espool
                                eqa = pool.tile(
                                    [P, (CHMAX if s == 0 else SMAX) * P], f16,
                                    tag="eqa" if s == 0 else "eqs")
                                nc.vector.tensor_tensor(
                                    out=eqa[:, : span * P].rearrange(
                                        "p (t d) -> p t d", d=P),
                                    in0=dstl_sb[:, gc0 + tA: gc0 + tB + 1
                                                ].to_broadcast([P, span, P]),
                                    in1=iota4_sb[:, s * P:(s + 1) * P].unsqueeze(
                                        1).broadcast_to([P, span, P]),
                                    op=mybir.AluOpType.is_equal,
                                )
                                eq_of[s] = (eqa, tA)
                        for (t, s) in plist:
                            b = b0 + s
                            if eq_mode == "batched":
                                eqa, tA = eq_of[s]
                                eq_ap = eqa[:, (t - tA) * P:(t - tA + 1) * P]
                            else:
                                eq = espool.tile([P, P], f16, tag="eq")
                                nc.vector.tensor_scalar(
                                    out=eq[:, :],
                                    in0=iota4_sb[:, s * P:(s + 1) * P],
                                    scalar1=dstl_sb[:, gc0 + t: gc0 + t + 1],
                                    scalar2=None,
                                    op0=mybir.AluOpType.is_equal,
                                )
                                eq_ap = eq[:, :]
                            nc.tensor.matmul(
                                aggs_ps[b][:, :],
                                lhsT=m[:, t * F:(t + 1) * F],
                                rhs=eq_ap,
                                start=False,
                                stop=last_pair[b] == (k, ci, t, s),
                            )
                    for b in range(b0, b1):
                        aggs = apool.tile([F, P], f32, tag="aggs")
                        nc.scalar.activation(
                            aggs[:, :], aggs_ps[b][:, :],
                            mybir.ActivationFunctionType.Copy,
                        )
                        if L < 2:
                            hp = ps_dense.tile([F, P], f32, tag="hp")
                            nc.tensor.matmul(
                                hp[:, :], lhsT=w_sb[L][:, :], rhs=aggs[:, :],
                                start=True, stop=True,
                            )
                            hT = apool.tile([F, P], f32, tag="hT")
                            nc.scalar.activation(
                                hT[:, :], hp[:, :],
                                mybir.ActivationFunctionType.Relu,
                                bias=b_sb[L][:, :],
                            )
                            tp = ps_tr.tile([P, F], f32, tag="tp")
                            nc.tensor.matmul(
                                tp[:, :], lhsT=hT[:, :], rhs=ident_sb[:, :],
                                is_transpose=True, start=True, stop=True,
                            )
                            hout = hpool.tile([P, F], f32, tag="hout")
                            nc.scalar.activation(
                                hout[:, :], tp[:, :],
                                mybir.ActivationFunctionType.Copy,
                            )
                            # hsc for the next layer: hout * self_coef (fp16)
                            nc.vector.tensor_scalar(
                                out=hsc_sb[L][:, b * F:(b + 1) * F],
                                in0=hout[:, :],
                                scalar1=scp_sb[:, b:b + 1],
                                scalar2=None,
                                op0=mybir.AluOpType.mult,
                            )
                            nc.sync.dma_start(
                                out=hloc[L][b * P: (b + 1) * P, :], in_=hout[:, :]
                            )
                        else:
                            yp = ps_dense.tile([1, P], f32, tag="hp", name="yp")
                            nc.tensor.matmul(
                                yp[:, :], lhsT=w_sb[2][:, :], rhs=aggs[:, :],
                                start=True, stop=True,
                            )
                            nc.scalar.activation(
                                y_sb[:, b * P: (b + 1) * P], yp[:, :],
                                mybir.ActivationFunctionType.Copy,
                            )
                if L < 2 and not skip_collective:
                    nc.gpsimd.collective_compute(
                        "AllGather",
                        mybir.AluOpType.bypass,
                        replica_groups=[list(range(C))],
                        ins=[hloc[L].ap().opt()],
                        outs=[htab[L].ap().opt()],
                    )
            nc.sync.dma_start(out=y_d[:, :], in_=y_sb[:, :])

    nc.compile()
    return nc


def make_in_maps(meta, W0, b0, W1, b1, W2):
    C = meta["C"]; F = meta["F"]; K = meta["K"]
    iota4 = np.tile(np.arange(meta["bpc"] * P), (P, 1)).astype(np.float16)
    id128 = np.eye(P, dtype=np.float16)
    common = dict(
        xt=meta["xt"],
        iota4=iota4,
        id128=id128,
        ident=np.eye(F, dtype=np.float32),
        w0=np.asarray(W0, np.float32),
        w1=np.asarray(W1, np.float32),
        w2=np.asarray(W2, np.float32).reshape(F, 1),
        b0=np.asarray(b0, np.float32).reshape(F, 1),
        b1=np.asarray(b1, np.float32).reshape(F, 1),
    )
    in_maps = []
    for c in range(C):
        im = dict(common)
        im["coef"] = meta["coef"][c]
        im["dstl"] = meta["dstl"][c]
        im["xsc"] = meta["xsc"][c]
        im["scp"] = meta["scp"][c]
        im["ncnt"] = meta["ncnt"][c].reshape(1, -1)
        for k in range(K):
            im[f"gix{k}"] = meta["gidx16"][k][c]
        in_maps.append(im)
    return in_maps


def assemble_output(meta, results, b2):
    C = meta["C"]
    rows_pc = meta["rows_pc"]
    ys = np.stack([np.asarray(results[c]["y"]).reshape(rows_pc) for c in range(C)])
    y = ys[meta["core_of"], meta["pos_of"]] + np.float32(np.asarray(b2).reshape(-1)[0])
    return y.astype(np.float32)


def kernel(x, edge_src, edge_dst, edge_weights, W0, b0, W1, b1, W2, b2,
           n_queues=4, trace=False):
    """Harness entry point: full inputs in, full [N] float32 output."""
    x = np.asarray(x)
    meta = preprocess(x, np.asarray(edge_src), np.asarray(edge_dst),
                      np.asarray(edge_weights))
    nc = build_nc(meta, n_queues=n_queues)
    in_maps = make_in_maps(meta, W0, b0, W1, b1, W2)
    last_err = None
    for attempt in range(3):
        try:
            res = bass_utils.run_bass_kernel_spmd(
                nc, in_maps, core_ids=list(range(meta["C"])), trace=trace
            )
            y = assemble_output(meta, res.results, b2)
            kernel.last_result = res
            kernel.last_nc = nc
            kernel.last_meta = meta
            kernel.last_in_maps = in_maps
            return y
        except Exception as e:  # transient accelerator failures: retry
            last_err = e
    raise last_err



# revision 3
# speedup vs baseline: 23.4181x; 23.4181x over previous
"""3-layer GCN (GCNConv x3) distributed over 8 NeuronCores — v2.

Differences from v1 (kernel.py):
- Self-loops leave the gather stream: per block, one PE matmul accumulates
  hsc = h_local * self_coef into the agg PSUM via an identity rhs (layer 0's
  hsc comes precomputed from the host as `xsc`; later layers build it on DVE
  from the block's hout).
- Tiles pack across block boundaries within a (window, chunk) group: a
  128-edge tile may span 2+ blocks.  dstl encodes chunk-relative ids
  (128*(b-b0)+d); one extra matmul per (tile, covered-block) pair, with the
  pair list unioned across cores so the SPMD program is identical everywhere
  (foreign pairs accumulate zeros).  Gather slot padding drops from ~30% to
  ~4%.
- Eq matrices are built batched: one DVE tensor_tensor per (window, chunk,
  block-slot) over the covered tile range, against an iota4 [P, 512] constant.
"""

import sys

sys.path.insert(0, "/opt/trn_rl_repo")

import numpy as np

from concourse import bass, bacc, mybir, tile
from concourse import bass_utils

P = 128
WIN = 32768  # int16 index window


def preprocess(x, edge_src, edge_dst, edge_weights, n_cores=8, bpc=4):
    N, F = x.shape
    E = edge_src.shape[0]
    C = n_cores

    w64 = edge_weights.astype(np.float64)
    deg = np.bincount(edge_dst, weights=w64, minlength=N) + 1.0
    dinv = 1.0 / np.sqrt(deg)
    norm = (dinv[edge_src] * w64 * dinv[edge_dst]).astype(np.float32)
    self_coef = (dinv * dinv).astype(np.float32)

    indeg = np.bincount(edge_dst, minlength=N)
    rounds = indeg + 1

    # deal nodes by descending degree: rank r -> core r%C, pos r//C
    order = np.argsort(-rounds, kind="stable")
    core_of = np.empty(N, np.int64)
    pos_of = np.empty(N, np.int64)
    r = np.arange(N)
    core_of[order] = r % C
    pos_of[order] = r // C

    npc = N // C
    B = (npc + P - 1) // P
    rows_pc = B * P
    rows_total = C * rows_pc
    pid = core_of * rows_pc + pos_of
    blk_of = pos_of // P
    K = (rows_total + WIN - 1) // WIN

    chunks = [(b, min(b + bpc, B)) for b in range(0, B, bpc)]
    NCH = len(chunks)
    ch_of_blk = np.zeros(B, np.int64)
    for ci, (b0, b1) in enumerate(chunks):
        ch_of_blk[b0:b1] = ci

    e_core = core_of[edge_dst]
    e_blk = blk_of[edge_dst]
    e_ch = ch_of_blk[e_blk]
    e_pid_src = pid[edge_src]
    e_win = e_pid_src // WIN
    e_lidx = (e_pid_src % WIN).astype(np.int32)
    e_d = (pos_of[edge_dst] % P).astype(np.int32)

    # sort by (core, window, chunk, block); edges of a group laid densely
    key = ((e_core * K + e_win) * B + e_blk)  # block implies chunk
    sort_e = np.argsort(key, kind="stable")
    cnt_ckb = np.bincount(key, minlength=C * K * B).reshape(C, K, B)

    # per (c,k,ci) group sizes, tiles per (k,ci) = max over cores
    cnt_ckc = np.zeros((C, K, NCH), np.int64)
    for ci, (b0, b1) in enumerate(chunks):
        cnt_ckc[:, :, ci] = cnt_ckb[:, :, b0:b1].sum(axis=2)
    Tkc = np.maximum.reduce(-(-cnt_ckc // P), axis=0)  # [K, NCH]
    Tkc = np.maximum(Tkc, 1)  # keep >=1 col per group for simplicity
    choff = np.zeros((K, NCH + 1), np.int64)
    choff[:, 1:] = np.cumsum(Tkc, axis=1)
    TOTk = choff[:, -1].copy()
    win_base = np.zeros(K + 1, np.int64)
    win_base[1:] = np.cumsum(TOTk)
    TOT = int(win_base[-1])

    # edge position within its (c,k,ci) group (block-sorted)
    gkey = (e_core * K + e_win) * NCH + e_ch
    gkey_s = gkey[sort_e]
    n_groups = C * K * NCH
    gcnt = np.bincount(gkey_s, minlength=n_groups)
    gfirst = np.zeros(n_groups + 1, np.int64)
    gfirst[1:] = np.cumsum(gcnt)
    jpos = np.arange(E) - gfirst[gkey_s]

    es = sort_e
    tile_rel = jpos // P
    slot = jpos % P
    col_w = choff[e_win[es], e_ch[es]] + tile_rel
    col_g = win_base[e_win[es]] + col_w
    # chunk-relative dst encoding
    b0_arr = np.array([c0 for (c0, _) in chunks])
    enc = (e_blk[es] - b0_arr[e_ch[es]]) * P + e_d[es]
    ecore = e_core[es]

    # pad indices are -1: trailing negatives are skipped by the gather DGE
    # (no descriptor emitted), with num_idxs_reg giving the per-core count
    gidx_flat = [np.full((C, max(1, int(TOTk[k])) * P), -1, np.int16)
                 for k in range(K)]
    coef = np.zeros((C, P, TOT), np.float32)
    dstl = np.full((C, P, TOT), 999.0, np.float32)

    ew = e_win[es]
    for k in range(K):
        m = ew == k
        gidx_flat[k][ecore[m], col_w[m] * P + slot[m]] = e_lidx[es][m].astype(
            np.int16
        )
    coef[ecore, slot, col_g] = norm[es]
    dstl[ecore, slot, col_g] = enc.astype(np.float32)

    # union matmul pair list: per (k,ci): sorted (t, s) pairs present in ANY
    # core, plus per-(b) last-pair bookkeeping
    pairs = {}
    for k in range(K):
        for ci, (b0, b1) in enumerate(chunks):
            pset = set()
            for c in range(C):
                cum = 0
                for s, b in enumerate(range(b0, b1)):
                    nb = int(cnt_ckb[c, k, b])
                    if nb == 0:
                        cum += 0
                        continue
                    t_lo = cum // P
                    t_hi = (cum + nb - 1) // P
                    for t in range(t_lo, t_hi + 1):
                        pset.add((t, s))
                    cum += nb
            pairs[(k, ci)] = sorted(pset)

    # per-core valid-index counts per gather call (call = (k, ci))
    assert (cnt_ckc >= 1).all(), "gather group with zero edges"
    ncnt = cnt_ckc.reshape(C, K * NCH).astype(np.int32)

    # per-window int16 index streams wrapped in 16 partitions, replicated x8
    gidx16 = []
    for k in range(K):
        nidx = gidx_flat[k].shape[1]
        w = gidx_flat[k].reshape(C, nidx // 16, 16).transpose(0, 2, 1)
        gidx16.append(np.tile(w, (1, 8, 1)).astype(np.int16))

    xt = np.zeros((rows_total, F), np.float32)
    xt[pid] = np.asarray(x, np.float32)

    # per-core hsc inputs: xsc[d, b*F+f] = x_local * self_coef (fp16), and
    # scp[d, b] = self_coef for on-device hsc builds in later layers
    xsc = np.zeros((C, P, B * F), np.float16)
    scp = np.zeros((C, P, B), np.float32)
    xl = xt.reshape(C, B, P, F)
    scl = np.zeros((C, rows_pc), np.float32)
    scl[core_of, pos_of] = self_coef
    scl = scl.reshape(C, B, P)
    for c in range(C):
        hs = xl[c] * scl[c][:, :, None]          # [B, P, F]
        xsc[c] = hs.transpose(1, 0, 2).reshape(P, B * F).astype(np.float16)
        scp[c] = scl[c].transpose(1, 0)           # [P, B]

    return dict(
        C=C, N=N, F=F, B=B, K=K, bpc=bpc, chunks=chunks,
        Tkc=Tkc, choff=choff, TOTk=[int(t) for t in TOTk],
        win_base=[int(w) for w in win_base], TOT=TOT, pairs=pairs,
        cnt_ckb=cnt_ckb,
        rows_pc=rows_pc, rows_total=rows_total,
        core_of=core_of, pos_of=pos_of, NCH=NCH,
        xt=xt, gidx16=gidx16, coef=coef, dstl=dstl, xsc=xsc, scp=scp,
        ncnt=ncnt,
    )


def build_nc(meta, skip_collective=False, scratch=16384, n_queues=4,
             eq_mode="batched", pad_skip=True, repeat=1):
    C = meta["C"]; F = meta["F"]; B = meta["B"]; K = meta["K"]
    chunks = meta["chunks"]; Tkc = meta["Tkc"]; choff = meta["choff"]
    TOTk = meta["TOTk"]; win_base = meta["win_base"]; TOT = meta["TOT"]
    pairs = meta["pairs"]
    rows_pc = meta["rows_pc"]; rows_total = meta["rows_total"]
    dt = mybir.dt
    f32 = dt.float32
    f16 = dt.float16

    CHMAX = int(max(Tkc[k][ci] for k in range(K) for ci in range(len(chunks))))
    # widest batched-eq span needed for block-slots s >= 1
    SMAX = 1
    for k in range(K):
        for ci in range(len(chunks)):
            for s in set(s_ for (_, s_) in pairs[(k, ci)] if s_ > 0):
                ts = [t for (t, s_) in pairs[(k, ci)] if s_ == s]
                SMAX = max(SMAX, max(ts) - min(ts) + 1)

    # last (k,ci,t,s) pair per block, for PSUM stop flags
    last_pair = {}
    for k in range(K):
        for ci, (b0, b1) in enumerate(chunks):
            for (t, s) in pairs[(k, ci)]:
                last_pair[b0 + s] = (k, ci, t, s)
    assert all(b in last_pair for b in range(B)), "block with no edge tiles"

    nc = bacc.Bacc("TRN2", target_bir_lowering=False, debug=False, num_devices=C,
                   dynamic_dma_scratch_size=scratch, num_swdge_queues=n_queues)

    xt = nc.dram_tensor("xt", [rows_total, F], f32, kind="ExternalInput").ap()
    gixd = [
        nc.dram_tensor(f"gix{k}", [P, max(1, TOTk[k]) * 8], dt.int16,
                       kind="ExternalInput").ap()
        for k in range(K)
    ]
    coef_d = nc.dram_tensor("coef", [P, TOT], f32, kind="ExternalInput").ap()
    dstl_d = nc.dram_tensor("dstl", [P, TOT], f32, kind="ExternalInput").ap()
    SBLK = meta["bpc"]
    iota4_d = nc.dram_tensor("iota4", [P, SBLK * P], f16,
                            kind="ExternalInput").ap()
    id128_d = nc.dram_tensor("id128", [P, P], f16, kind="ExternalInput").ap()
    ident_d = nc.dram_tensor("ident", [F, F], f32, kind="ExternalInput").ap()
    xsc_d = nc.dram_tensor("xsc", [P, B * F], f16, kind="ExternalInput").ap()
    scp_d = nc.dram_tensor("scp", [P, B], f32, kind="ExternalInput").ap()
    NCALL = meta["ncnt"].shape[1]
    ncnt_d = nc.dram_tensor("ncnt", [1, NCALL], dt.int32, kind="ExternalInput").ap()
    w_d = [
        nc.dram_tensor(f"w{i}", [F, F if i < 2 else 1], f32, kind="ExternalInput").ap()
        for i in range(3)
    ]
    b_d = [
        nc.dram_tensor(f"b{i}", [F, 1], f32, kind="ExternalInput").ap()
        for i in range(2)
    ]
    y_d = nc.dram_tensor("y", [1, rows_pc], f32, kind="ExternalOutput").ap()

    hloc = [nc.dram_tensor(f"hloc{i}", [rows_pc, F], f32) for i in range(2)]
    htab = [
        nc.dram_tensor(f"htab{i}", [rows_total, F], f32, addr_space="Shared")
        for i in range(2)
    ]

    nbuf = 3 if CHMAX <= 32 else 2
    with tile.TileContext(nc) as tc:
        with (
            tc.tile_pool(name="const", bufs=1) as cpool,
            tc.tile_pool(name="gather", bufs=nbuf) as gpool,
            tc.tile_pool(name="msgs", bufs=nbuf) as mpool,
            tc.tile_pool(name="eqp", bufs=nbuf) as epool,
            tc.tile_pool(name="eqs", bufs=6) as espool,
            tc.tile_pool(name="aggs", bufs=3) as apool,
            tc.tile_pool(name="hout", bufs=3) as hpool,
            tc.tile_pool(name="psum_agg", bufs=4, space="PSUM") as ps_agg,
            tc.tile_pool(name="psum_dense", bufs=2, space="PSUM") as ps_dense,
            tc.tile_pool(name="psum_tr", bufs=2, space="PSUM") as ps_tr,
        ):
            gix_sb = [
                cpool.tile([P, max(1, TOTk[k]) * 8], dt.int16, tag=f"gix{k}",
                           name=f"gix{k}sb")
                for k in range(K)
            ]
            coef_sb = cpool.tile([P, TOT], f32, tag="coef")
            dstl_sb = cpool.tile([P, TOT], f32, tag="dstl")
            iota4_sb = cpool.tile([P, SBLK * P], f16, tag="iota4")
            id128_sb = cpool.tile([P, P], f16, tag="id128")
            ident_sb = cpool.tile([F, F], f32, tag="ident")
            xsc_sb = cpool.tile([P, B * F], f16, tag="xsc")
            scp_sb = cpool.tile([P, B], f32, tag="scp")
            ncnt_sb = cpool.tile([1, NCALL], dt.int32, tag="ncnt")
            hsc_sb = [cpool.tile([P, B * F], f16, tag=f"hsc{i}",
                                 name=f"hsc{i}sb") for i in range(2)]
            w_sb = [cpool.tile([F, F if i < 2 else 1], f32, tag=f"w{i}",
                               name=f"w{i}sb") for i in range(3)]
            b_sb = [cpool.tile([F, 1], f32, tag=f"b{i}", name=f"b{i}sb")
                    for i in range(2)]
            y_sb = cpool.tile([1, rows_pc], f32, tag="ysb")

            for k in range(K):
                nc.sync.dma_start(out=gix_sb[k][:, :], in_=gixd[k][:, :])
            nc.sync.dma_start(out=coef_sb[:, :], in_=coef_d[:, :])
            nc.sync.dma_start(out=dstl_sb[:, :], in_=dstl_d[:, :])
            nc.sync.dma_start(out=iota4_sb[:, :], in_=iota4_d[:, :])
            nc.sync.dma_start(out=id128_sb[:, :], in_=id128_d[:, :])
            nc.sync.dma_start(out=ident_sb[:, :], in_=ident_d[:, :])
            nc.sync.dma_start(out=xsc_sb[:, :], in_=xsc_d[:, :])
            nc.sync.dma_start(out=scp_sb[:, :], in_=scp_d[:, :])
            nc.sync.dma_start(out=ncnt_sb[:, :], in_=ncnt_d[:, :])
            gcnt_reg = (nc.alloc_register(mybir.EngineType.Pool, "gcnt")
                        if pad_skip else None)
            for i in range(3):
                nc.sync.dma_start(out=w_sb[i][:, :], in_=w_d[i][:, :])
            for i in range(2):
                nc.sync.dma_start(out=b_sb[i][:, :], in_=b_d[i][:, :])

            call_no = 0
            for rep in range(repeat):
             for L in range(3):
                table = [xt, htab[0][:, :], htab[1][:, :]][L]
                hsc_cur = [xsc_sb, hsc_sb[0], hsc_sb[1]][L]
                for ci, (b0, b1) in enumerate(chunks):
                    aggs_ps = {}
                    for b in range(b0, b1):
                        aggs_ps[b] = ps_agg.tile([F, P], f32, tag="agg",
                                                 name=f"agg{L}_{b}")
                        # self-loop: agg += hsc_b^T via identity rhs
                        nc.tensor.matmul(
                            aggs_ps[b][:, :],
                            lhsT=hsc_cur[:, b * F:(b + 1) * F],
                            rhs=id128_sb[:, :],
                            start=True,
                            stop=False,
                        )
                    for k in range(K):
                        cw0 = int(choff[k][ci]); cw1 = int(choff[k][ci + 1])
                        cols = cw1 - cw0
                        gc0 = win_base[k] + cw0
                        gc1 = win_base[k] + cw1
                        g = gpool.tile([P, CHMAX * F], f32, tag="g")
                        ci_call = k * meta["NCH"] + ci
                        if pad_skip:
                            nc.gpsimd.reg_load(
                                gcnt_reg, ncnt_sb[0:1, ci_call:ci_call + 1])
                            nreg = gcnt_reg
                        else:
                            nreg = cols * P
                        nc.gpsimd.dma_gather(
                            out_ap=g[:, : cols * F].rearrange(
                                "p (t f) -> p t f", f=F),
                            in_ap=table[k * WIN: min((k + 1) * WIN, rows_total), :],
                            idxs_ap=gix_sb[k][:, cw0 * 8: cw1 * 8],
                            num_idxs=cols * P,
                            num_idxs_reg=nreg,
                            elem_size=F,
                            single_packet=False,
                            queue_num=call_no % n_queues,
                        )
                        call_no += 1
                        m = mpool.tile([P, CHMAX * F], f16, tag="m")
                        nc.vector.tensor_tensor(
                            out=m[:, : cols * F].rearrange("p (t f) -> p t f", f=F),
                            in0=g[:, : cols * F].rearrange("p (t f) -> p t f", f=F),
                            in1=coef_sb[:, gc0:gc1].to_broadcast([P, cols, F]),
                            op=mybir.AluOpType.mult,
                        )
                        plist = pairs[(k, ci)]
                        eq_of = {}
                        if eq_mode == "batched":
                            # batched eq per block-slot s over its tile range
                            for s in sorted(set(s_ for (_, s_) in plist)):
                                ts = [t for (t, s_) in plist if s_ == s]
                                tA, tB = min(ts), max(ts)
                                span = tB - tA + 1
                                pool = epool if s == 0 else espool
                                eqa = pool.tile(
                                    [P, (CHMAX if s == 0 else SMAX) * P], f16,
                                    tag="eqa" if s == 0 else "eqs")
                                nc.vector.tensor_tensor(
                                    out=eqa[:, : span * P].rearrange(
                                        "p (t d) -> p t d", d=P),
                                    in0=dstl_sb[:, gc0 + tA: gc0 + tB + 1
                                                ].to_broadcast([P, span, P]),
                                    in1=iota4_sb[:, s * P:(s + 1) * P].unsqueeze(
                                        1).broadcast_to([P, span, P]),
                                    op=mybir.AluOpType.is_equal,
                                )
                                eq_of[s] = (eqa, tA)
                        for (t, s) in plist:
                            b = b0 + s
                            if eq_mode == "batched":
                                eqa, tA = eq_of[s]
                                eq_ap = eqa[:, (t - tA) * P:(t - tA + 1) * P]
                            else:
                                eq = espool.tile([P, P], f16, tag="eq")
                                nc.vector.tensor_scalar(
                                    out=eq[:, :],
                                    in0=iota4_sb[:, s * P:(s + 1) * P],
                                    scalar1=dstl_sb[:, gc0 + t: gc0 + t + 1],
                                    scalar2=None,
                                    op0=mybir.AluOpType.is_equal,
                                )
                                eq_ap = eq[:, :]
                            nc.tensor.matmul(
                                aggs_ps[b][:, :],
                                lhsT=m[:, t * F:(t + 1) * F],
                                rhs=eq_ap,
                                start=False,
                                stop=last_pair[b] == (k, ci, t, s),
                            )
                    for b in range(b0, b1):
                        aggs = apool.tile([F, P], f32, tag="aggs")
                        nc.scalar.activation(
                            aggs[:, :], aggs_ps[b][:, :],
                            mybir.ActivationFunctionType.Copy,
                        )
                        if L < 2:
                            hp = ps_dense.tile([F, P], f32, tag="hp")
                            nc.tensor.matmul(
                                hp[:, :], lhsT=w_sb[L][:, :], rhs=aggs[:, :],
                                start=True, stop=True,
                            )
                            hT = apool.tile([F, P], f32, tag="hT")
                            nc.scalar.activation(
                                hT[:, :], hp[:, :],
                                mybir.ActivationFunctionType.Relu,
                                bias=b_sb[L][:, :],
                            )
                            tp = ps_tr.tile([P, F], f32, tag="tp")
                            nc.tensor.matmul(
                                tp[:, :], lhsT=hT[:, :], rhs=ident_sb[:, :],
                                is_transpose=True, start=True, stop=True,
                            )
                            hout = hpool.tile([P, F], f32, tag="hout")
                            nc.scalar.activation(
                                hout[:, :], tp[:, :],
                                mybir.ActivationFunctionType.Copy,
                            )
                            # hsc for the next layer: hout * self_coef (fp16)
                            nc.vector.tensor_scalar(
                                out=hsc_sb[L][:, b * F:(b + 1) * F],
                                in0=hout[:, :],
                                scalar1=scp_sb[:, b:b + 1],
                                scalar2=None,
                                op0=mybir.AluOpType.mult,
                            )
                            nc.sync.dma_start(
                                out=hloc[L][b * P: (b + 1) * P, :], in_=hout[:, :]
                            )
                        else:
                            yp = ps_dense.tile([1, P], f32, tag="hp", name="yp")
                            nc.tensor.matmul(
                                yp[:, :], lhsT=w_sb[2][:, :], rhs=aggs[:, :],
                                start=True, stop=True,
                            )
                            nc.scalar.activation(
                                y_sb[:, b * P: (b + 1) * P], yp[:, :],
                                mybir.ActivationFunctionType.Copy,
                            )
                if L < 2 and not skip_collective:
                    nc.gpsimd.collective_compute(
                        "AllGather",
                        mybir.AluOpType.bypass,
                        replica_groups=[list(range(C))],
                        ins=[hloc[L].ap().opt()],
                        outs=[htab[L].ap().opt()],
                    )
            nc.sync.dma_start(out=y_d[:, :], in_=y_sb[:, :])

    nc.compile()
    return nc


def make_in_maps(meta, W0, b0, W1, b1, W2):
    C = meta["C"]; F = meta["F"]; K = meta["K"]
    iota4 = np.tile(np.arange(meta["bpc"] * P), (P, 1)).astype(np.float16)
    id128 = np.eye(P, dtype=np.float16)
    common = dict(
        xt=meta["xt"],
        iota4=iota4,
        id128=id128,
        ident=np.eye(F, dtype=np.float32),
        w0=np.asarray(W0, np.float32),
        w1=np.asarray(W1, np.float32),
        w2=np.asarray(W2, np.float32).reshape(F, 1),
        b0=np.asarray(b0, np.float32).reshape(F, 1),
        b1=np.asarray(b1, np.float32).reshape(F, 1),
    )
    in_maps = []
    for c in range(C):
        im = dict(common)
        im["coef"] = meta["coef"][c]
        im["dstl"] = meta["dstl"][c]
        im["xsc"] = meta["xsc"][c]
        im["scp"] = meta["scp"][c]
        im["ncnt"] = meta["ncnt"][c].reshape(1, -1)
        for k in range(K):
            im[f"gix{k}"] = meta["gidx16"][k][c]
        in_maps.append(im)
    return in_maps


def assemble_output(meta, results, b2):
    C = meta["C"]
    rows_pc = meta["rows_pc"]
    ys = np.stack([np.asarray(results[c]["y"]).reshape(rows_pc) for c in range(C)])
    y = ys[meta["core_of"], meta["pos_of"]] + np.float32(np.asarray(b2).reshape(-1)[0])
    return y.astype(np.float32)


def kernel(x, edge_src, edge_dst, edge_weights, W0, b0, W1, b1, W2, b2,
           n_queues=4, trace=False):
    """Harness entry point: full inputs in, full [N] float32 output."""
    x = np.asarray(x)
    meta = preprocess(x, np.asarray(edge_src), np.asarray(edge_dst),
                      np.asarray(edge_weights))
    nc = build_nc(meta, n_queues=n_queues)
    in_maps = make_in_maps(meta, W0, b0, W1, b1, W2)
    last_err = None
    for attempt in range(3):
        try:
            res = bass_utils.run_bass_kernel_spmd(
                nc, in_maps, core_ids=list(range(meta["C"])), trace=trace
            )
            y = assemble_output(meta, res.results, b2)
            kernel.last_result = res
            kernel.last_nc = nc
            kernel.last_meta = meta
            kernel.last_in_maps = in_maps
            return y
        except Exception as e:  # transient accelerator failures: retry
            last_err = e
    raise last_err



# revision 20
# speedup vs baseline: 24.4656x; 1.0447x over previous
"""3-layer GCN (GCNConv x3) distributed over 8 NeuronCores — v2.

Differences from v1 (kernel.py):
- Self-loops leave the gather stream: per block, one PE matmul accumulates
  hsc = h_local * self_coef into the agg PSUM via an identity rhs (layer 0's
  hsc comes precomputed from the host as `xsc`; later layers build it on DVE
  from the block's hout).
- Tiles pack across block boundaries within a (window, chunk) group: a
  128-edge tile may span 2+ blocks.  dstl encodes chunk-relative ids
  (128*(b-b0)+d); one extra matmul per (tile, covered-block) pair, with the
  pair list unioned across cores so the SPMD program is identical everywhere
  (foreign pairs accumulate zeros).  Gather slot padding drops from ~30% to
  ~4%.
- Eq matrices are built batched: one DVE tensor_tensor per (window, chunk,
  block-slot) over the covered tile range, against an iota4 [P, 512] constant.
"""

import sys

sys.path.insert(0, "/opt/trn_rl_repo")

import numpy as np

from concourse import bass, bacc, mybir, tile
from concourse import bass_utils

P = 128
WIN = 32768  # int16 index window


def preprocess(x, edge_src, edge_dst, edge_weights, n_cores=8, bpc=4):
    N, F = x.shape
    E = edge_src.shape[0]
    C = n_cores

    w64 = edge_weights.astype(np.float64)
    deg = np.bincount(edge_dst, weights=w64, minlength=N) + 1.0
    dinv = 1.0 / np.sqrt(deg)
    norm = (dinv[edge_src] * w64 * dinv[edge_dst]).astype(np.float32)
    self_coef = (dinv * dinv).astype(np.float32)

    indeg = np.bincount(edge_dst, minlength=N)
    rounds = indeg + 1

    # deal nodes by descending degree: rank r -> core r%C, pos r//C
    order = np.argsort(-rounds, kind="stable")
    core_of = np.empty(N, np.int64)
    pos_of = np.empty(N, np.int64)
    r = np.arange(N)
    core_of[order] = r % C
    pos_of[order] = r // C

    npc = N // C
    B = (npc + P - 1) // P
    rows_pc = B * P
    rows_total = C * rows_pc
    pid = core_of * rows_pc + pos_of
    blk_of = pos_of // P
    K = (rows_total + WIN - 1) // WIN

    chunks = [(b, min(b + bpc, B)) for b in range(0, B, bpc)]
    NCH = len(chunks)
    ch_of_blk = np.zeros(B, np.int64)
    for ci, (b0, b1) in enumerate(chunks):
        ch_of_blk[b0:b1] = ci

    e_core = core_of[edge_dst]
    e_blk = blk_of[edge_dst]
    e_ch = ch_of_blk[e_blk]
    e_pid_src = pid[edge_src]
    e_win = e_pid_src // WIN
    e_lidx = (e_pid_src % WIN).astype(np.int32)
    e_d = (pos_of[edge_dst] % P).astype(np.int32)

    # sort by (core, window, chunk, block); edges of a group laid densely
    key = ((e_core * K + e_win) * B + e_blk)  # block implies chunk
    sort_e = np.argsort(key, kind="stable")
    cnt_ckb = np.bincount(key, minlength=C * K * B).reshape(C, K, B)

    # per (c,k,ci) group sizes, tiles per (k,ci) = max over cores
    cnt_ckc = np.zeros((C, K, NCH), np.int64)
    for ci, (b0, b1) in enumerate(chunks):
        cnt_ckc[:, :, ci] = cnt_ckb[:, :, b0:b1].sum(axis=2)
    Tkc = np.maximum.reduce(-(-cnt_ckc // P), axis=0)  # [K, NCH]
    Tkc = np.maximum(Tkc, 1)  # keep >=1 col per group for simplicity
    choff = np.zeros((K, NCH + 1), np.int64)
    choff[:, 1:] = np.cumsum(Tkc, axis=1)
    TOTk = choff[:, -1].copy()
    win_base = np.zeros(K + 1, np.int64)
    win_base[1:] = np.cumsum(TOTk)
    TOT = int(win_base[-1])

    # edge position within its (c,k,ci) group (block-sorted)
    gkey = (e_core * K + e_win) * NCH + e_ch
    gkey_s = gkey[sort_e]
    n_groups = C * K * NCH
    gcnt = np.bincount(gkey_s, minlength=n_groups)
    gfirst = np.zeros(n_groups + 1, np.int64)
    gfirst[1:] = np.cumsum(gcnt)
    jpos = np.arange(E) - gfirst[gkey_s]

    es = sort_e
    tile_rel = jpos // P
    slot = jpos % P
    col_w = choff[e_win[es], e_ch[es]] + tile_rel
    col_g = win_base[e_win[es]] + col_w
    # chunk-relative dst encoding
    b0_arr = np.array([c0 for (c0, _) in chunks])
    enc = (e_blk[es] - b0_arr[e_ch[es]]) * P + e_d[es]
    ecore = e_core[es]

    # pad indices are -1: trailing negatives are skipped by the gather DGE
    # (no descriptor emitted), with num_idxs_reg giving the per-core count
    gidx_flat = [np.full((C, max(1, int(TOTk[k])) * P), -1, np.int16)
                 for k in range(K)]
    coef = np.zeros((C, P, TOT), np.float32)
    dstl = np.full((C, P, TOT), 999.0, np.float16)

    ew = e_win[es]
    for k in range(K):
        m = ew == k
        gidx_flat[k][ecore[m], col_w[m] * P + slot[m]] = e_lidx[es][m].astype(
            np.int16
        )
    coef[ecore, slot, col_g] = norm[es]
    dstl[ecore, slot, col_g] = enc.astype(np.float16)

    # union matmul pair list: per (k,ci): sorted (t, s) pairs present in ANY
    # core, plus per-(b) last-pair bookkeeping
    pairs = {}
    for k in range(K):
        for ci, (b0, b1) in enumerate(chunks):
            pset = set()
            for c in range(C):
                cum = 0
                for s, b in enumerate(range(b0, b1)):
                    nb = int(cnt_ckb[c, k, b])
                    if nb == 0:
                        cum += 0
                        continue
                    t_lo = cum // P
                    t_hi = (cum + nb - 1) // P
                    for t in range(t_lo, t_hi + 1):
                        pset.add((t, s))
                    cum += nb
            pairs[(k, ci)] = sorted(pset)

    # per-core valid-index counts per gather call (call = (k, ci))
    assert (cnt_ckc >= 1).all(), "gather group with zero edges"
    ncnt = cnt_ckc.reshape(C, K * NCH).astype(np.int32)

    # per-window int16 index streams wrapped in 16 partitions, replicated x8
    gidx16 = []
    for k in range(K):
        nidx = gidx_flat[k].shape[1]
        w = gidx_flat[k].reshape(C, nidx // 16, 16).transpose(0, 2, 1)
        gidx16.append(np.tile(w, (1, 8, 1)).astype(np.int16))

    xt = np.zeros((rows_total, F), np.float32)
    xt[pid] = np.asarray(x, np.float32)

    # per-core hsc inputs: xsc[d, b*F+f] = x_local * self_coef (fp16), and
    # scp[d, b] = self_coef for on-device hsc builds in later layers
    xsc = np.zeros((C, P, B * F), np.float16)
    scp = np.zeros((C, P, B), np.float32)
    xl = xt.reshape(C, B, P, F)
    scl = np.zeros((C, rows_pc), np.float32)
    scl[core_of, pos_of] = self_coef
    scl = scl.reshape(C, B, P)
    for c in range(C):
        hs = xl[c] * scl[c][:, :, None]          # [B, P, F]
        xsc[c] = hs.transpose(1, 0, 2).reshape(P, B * F).astype(np.float16)
        scp[c] = scl[c].transpose(1, 0)           # [P, B]

    return dict(
        C=C, N=N, F=F, B=B, K=K, bpc=bpc, chunks=chunks,
        Tkc=Tkc, choff=choff, TOTk=[int(t) for t in TOTk],
        win_base=[int(w) for w in win_base], TOT=TOT, pairs=pairs,
        cnt_ckb=cnt_ckb,
        rows_pc=rows_pc, rows_total=rows_total,
        core_of=core_of, pos_of=pos_of, NCH=NCH,
        xt=xt, gidx16=gidx16, coef=coef, dstl=dstl, xsc=xsc, scp=scp,
        ncnt=ncnt,
    )


def eq_widths(meta):
    """CHMAX: max tile-columns of any gather group; SMAX: max eq span over
    every (group, block-slot) — the iotaT per-slot expansion width."""
    Tkc = meta["Tkc"]; chunks = meta["chunks"]; K = meta["K"]
    pairs = meta["pairs"]
    CHMAX = int(max(Tkc[k][ci] for k in range(K) for ci in range(len(chunks))))
    SMAX = 1
    for k in range(K):
        for ci in range(len(chunks)):
            for s in set(s_ for (_, s_) in pairs[(k, ci)]):
                ts = [t for (t, s_) in pairs[(k, ci)] if s_ == s]
                SMAX = max(SMAX, max(ts) - min(ts) + 1)
    return CHMAX, SMAX


def build_nc(meta, skip_collective=False, scratch=16384, n_queues=4,
             eq_mode="batched", pad_skip=True, repeat=1, gather_frac=1.0,
             eq_cycle=("dve",), mult_cycle=("dve",)):
    # gather_frac < 1 emits gathers over only a prefix of each call's columns
    # (timing experiments only: downstream data is garbage, ncnt input must be
    # recomputed to match -- see probe scripts).
    # eq_cycle / mult_cycle: engines ("dve"|"pool"|"act") assigned round-robin
    # to eq-matrix builds and message-coef multiplies, to spread DVE load.
    C = meta["C"]; F = meta["F"]; B = meta["B"]; K = meta["K"]
    chunks = meta["chunks"]; Tkc = meta["Tkc"]; choff = meta["choff"]
    TOTk = meta["TOTk"]; win_base = meta["win_base"]; TOT = meta["TOT"]
    pairs = meta["pairs"]
    rows_pc = meta["rows_pc"]; rows_total = meta["rows_total"]
    dt = mybir.dt
    f32 = dt.float32
    f16 = dt.float16

    CHMAX, SMAX = eq_widths(meta)

    # last (k,ci,t,s) pair per block and per chunk, for PSUM stop flags
    last_pair = {}
    chunk_last = {}
    for k in range(K):
        for ci, (b0, b1) in enumerate(chunks):
            for (t, s) in pairs[(k, ci)]:
                last_pair[b0 + s] = (k, ci, t, s)
                chunk_last[ci] = (k, ci, t, s)
    assert all(b in last_pair for b in range(B)), "block with no edge tiles"

    nc = bacc.Bacc("TRN2", target_bir_lowering=False, debug=False, num_devices=C,
                   dynamic_dma_scratch_size=scratch, num_swdge_queues=n_queues)

    xt = nc.dram_tensor("xt", [rows_total, F], f32, kind="ExternalInput").ap()
    gixd = [
        nc.dram_tensor(f"gix{k}", [P, max(1, TOTk[k]) * 8], dt.int16,
                       kind="ExternalInput").ap()
        for k in range(K)
    ]
    coef_d = nc.dram_tensor("coef", [P, TOT], f32, kind="ExternalInput").ap()
    dstl_d = nc.dram_tensor("dstl", [P, TOT], f16, kind="ExternalInput").ap()
    SBLK = meta["bpc"]
    IOTW = SBLK * SMAX
    iotaT_d = nc.dram_tensor("iotaT", [P, P * IOTW], f16,
                             kind="ExternalInput").ap()
    id128_d = nc.dram_tensor("id128", [P, P], f16, kind="ExternalInput").ap()
    ident_d = nc.dram_tensor("ident", [F, F], f32, kind="ExternalInput").ap()
    xsc_d = nc.dram_tensor("xsc", [P, B * F], f16, kind="ExternalInput").ap()
    scp_d = nc.dram_tensor("scp", [P, B], f32, kind="ExternalInput").ap()
    NCALL = meta["ncnt"].shape[1]
    ncnt_d = nc.dram_tensor("ncnt", [1, NCALL], dt.int32, kind="ExternalInput").ap()
    w_d = [
        nc.dram_tensor(f"w{i}", [F, F if i < 2 else 1], f32, kind="ExternalInput").ap()
        for i in range(3)
    ]
    b_d = [
        nc.dram_tensor(f"b{i}", [F, 1], f32, kind="ExternalInput").ap()
        for i in range(2)
    ]
    y_d = nc.dram_tensor("y", [1, rows_pc], f32, kind="ExternalOutput").ap()

    hloc = [nc.dram_tensor(f"hloc{i}", [rows_pc, F], f32) for i in range(2)]
    htab = [
        nc.dram_tensor(f"htab{i}", [rows_total, F], f32, addr_space="Shared")
        for i in range(2)
    ]

    nbuf = 3 if CHMAX <= 32 else 2
    with tile.TileContext(nc) as tc:
        with (
            tc.tile_pool(name="const", bufs=1) as cpool,
            tc.tile_pool(name="gather", bufs=nbuf) as gpool,
            tc.tile_pool(name="msgs", bufs=nbuf) as mpool,
            tc.tile_pool(name="eqs", bufs=8) as espool,
            tc.tile_pool(name="aggs", bufs=3) as apool,
            tc.tile_pool(name="hout", bufs=3) as hpool,
            tc.tile_pool(name="psum_agg", bufs=4, space="PSUM") as ps_agg,
            tc.tile_pool(name="psum_dense", bufs=2, space="PSUM") as ps_dense,
            tc.tile_pool(name="psum_tr", bufs=2, space="PSUM") as ps_tr,
        ):
            gix_sb = [
                cpool.tile([P, max(1, TOTk[k]) * 8], dt.int16, tag=f"gix{k}",
                           name=f"gix{k}sb")
                for k in range(K)
            ]
            coef_sb = cpool.tile([P, TOT], f32, tag="coef")
            dstl_sb = cpool.tile([P, TOT], f16, tag="dstl")
            iotaT_sb = cpool.tile([P, P * IOTW], f16, tag="iotaT")
            id128_sb = cpool.tile([P, P], f16, tag="id128")
            ident_sb = cpool.tile([F, F], f32, tag="ident")
            xsc_sb = cpool.tile([P, B * F], f16, tag="xsc")
            scp_sb = cpool.tile([P, B], f32, tag="scp")
            ncnt_sb = cpool.tile([1, NCALL], dt.int32, tag="ncnt")
            hsc_sb = [cpool.tile([P, B * F], f16, tag=f"hsc{i}",
                                 name=f"hsc{i}sb") for i in range(2)]
            w_sb = [cpool.tile([F, F if i < 2 else 1], f32, tag=f"w{i}",
                               name=f"w{i}sb") for i in range(3)]
            b_sb = [cpool.tile([F, 1], f32, tag=f"b{i}", name=f"b{i}sb")
                    for i in range(2)]
            y_sb = cpool.tile([1, rows_pc], f32, tag="ysb")

            for k in range(K):
                nc.sync.dma_start(out=gix_sb[k][:, :], in_=gixd[k][:, :])
            nc.sync.dma_start(out=coef_sb[:, :], in_=coef_d[:, :])
            nc.sync.dma_start(out=dstl_sb[:, :], in_=dstl_d[:, :])
            nc.sync.dma_start(out=iotaT_sb[:, :], in_=iotaT_d[:, :])
            nc.sync.dma_start(out=id128_sb[:, :], in_=id128_d[:, :])
            nc.sync.dma_start(out=ident_sb[:, :], in_=ident_d[:, :])
            nc.sync.dma_start(out=xsc_sb[:, :], in_=xsc_d[:, :])
            nc.sync.dma_start(out=scp_sb[:, :], in_=scp_d[:, :])
            nc.sync.dma_start(out=ncnt_sb[:, :], in_=ncnt_d[:, :])
            gcnt_reg = (nc.alloc_register(mybir.EngineType.Pool, "gcnt")
                        if pad_skip else None)
            for i in range(3):
                nc.sync.dma_start(out=w_sb[i][:, :], in_=w_d[i][:, :])
            for i in range(2):
                nc.sync.dma_start(out=b_sb[i][:, :], in_=b_d[i][:, :])

            call_no = 0
            eq_no = 0
            mult_no = 0
            for rep in range(repeat):
             for L in range(3):
                table = [xt, htab[0][:, :], htab[1][:, :]][L]
                hsc_cur = [xsc_sb, hsc_sb[0], hsc_sb[1]][L]
                for ci, (b0, b1) in enumerate(chunks):
                    # one PSUM bank holds the whole chunk's agg [F, bpc*P]
                    agg_ps = ps_agg.tile([F, SBLK * P], f32, tag="agg",
                                         name=f"agg{L}_{ci}")
                    aggs_ps = {b: agg_ps[:, (b - b0) * P:(b - b0 + 1) * P]
                               for b in range(b0, b1)}
                    for b in range(b0, b1):
                        # self-loop: agg += hsc_b^T via identity rhs.  start
                        # only on the first matmul of the chunk: PSUM zero
                        # regions are bank-wide, and the whole chunk shares
                        # one bank.
                        nc.tensor.matmul(
                            aggs_ps[b],
                            lhsT=hsc_cur[:, b * F:(b + 1) * F],
                            rhs=id128_sb[:, :],
                            start=(b == b0),
                            stop=False,
                        )
                    for k in range(K):
                        cw0 = int(choff[k][ci]); cw1 = int(choff[k][ci + 1])
                        cols = cw1 - cw0
                        gc0 = win_base[k] + cw0
                        gc1 = win_base[k] + cw1
                        g = gpool.tile([P, CHMAX * F], f32, tag="g")
                        ci_call = k * meta["NCH"] + ci
                        if pad_skip:
                            nc.gpsimd.reg_load(
                                gcnt_reg, ncnt_sb[0:1, ci_call:ci_call + 1])
                            nreg = gcnt_reg
                        else:
                            nreg = cols * P
                        cols_g = cols if gather_frac >= 1.0 else max(
                            1, int(cols * gather_frac))
                        nc.gpsimd.dma_gather(
                            out_ap=g[:, : cols_g * F].rearrange(
                                "p (t f) -> p t f", f=F),
                            in_ap=table[k * WIN: min((k + 1) * WIN, rows_total), :],
                            idxs_ap=gix_sb[k][:, cw0 * 8: (cw0 + cols_g) * 8],
                            num_idxs=cols_g * P,
                            num_idxs_reg=nreg,
                            elem_size=F,
                            single_packet=False,
                            queue_num=call_no % n_queues,
                        )
                        call_no += 1
                        m = mpool.tile([P, CHMAX * F], f16, tag="m")
                        meng = mult_cycle[mult_no % len(mult_cycle)]
                        mult_no += 1
                        if meng == "act":
                            # per-tile Activation copy with per-partition scale
                            for t in range(cols):
                                nc.scalar.activation(
                                    m[:, t * F:(t + 1) * F],
                                    g[:, t * F:(t + 1) * F],
                                    mybir.ActivationFunctionType.Copy,
                                    scale=coef_sb[:, gc0 + t: gc0 + t + 1],
                                )
                        else:
                            eng = nc.vector if meng == "dve" else nc.gpsimd
                            eng.tensor_tensor(
                                out=m[:, : cols * F].rearrange(
                                    "p (t f) -> p t f", f=F),
                                in0=g[:, : cols * F].rearrange(
                                    "p (t f) -> p t f", f=F),
                                in1=coef_sb[:, gc0:gc1].to_broadcast([P, cols, F]),
                                op=mybir.AluOpType.mult,
                            )
                        plist = pairs[(k, ci)]
                        # Transposed batched eq build: out[p, d, t] so that
                        # dstl (varying along t) rides the contiguous last
                        # axis and iota comes pre-expanded along t (constant
                        # input).  All operands 2-byte stride-1 last dim ->
                        # DVE fast mode (~3x).  Matmuls then read strided
                        # rhs slices eqT[:, :, t].
                        eq_of = {}
                        for s in sorted(set(s_ for (_, s_) in plist)):
                            ts = [t for (t, s_) in plist if s_ == s]
                            tA, tB = min(ts), max(ts)
                            span = tB - tA + 1
                            eqa = espool.tile([P, SMAX * P], f16,
                                                  tag="eqs")
                            eeng = eq_cycle[eq_no % len(eq_cycle)]
                            eq_no += 1
                            eng = nc.vector if eeng == "dve" else nc.gpsimd
                            eng.tensor_tensor(
                                out=eqa[:, : P * span].rearrange(
                                    "p (d t) -> p d t", t=span),
                                in0=dstl_sb[:, gc0 + tA: gc0 + tB + 1
                                            ].unsqueeze(1).broadcast_to(
                                                [P, P, span]),
                                in1=iotaT_sb[:, P * s * SMAX:
                                         P * (s + 1) * SMAX].rearrange(
                                "p (d t) -> p d t", t=SMAX)[:, :, :span],
                                op=mybir.AluOpType.is_equal,
                            )
                            eq_of[s] = (eqa, tA, span)
                        for (t, s) in plist:
                            b = b0 + s
                            eqa, tA, span = eq_of[s]
                            eq_ap = eqa[:, : P * span].rearrange(
                                "p (d t) -> p d t", t=span)[:, :, t - tA]
                            nc.tensor.matmul(
                                aggs_ps[b],
                                lhsT=m[:, t * F:(t + 1) * F],
                                rhs=eq_ap,
                                start=False,
                                stop=chunk_last[ci] == (k, ci, t, s),
                            )
                    for b in range(b0, b1):
                        aggs = apool.tile([F, P], f32, tag="aggs")
                        nc.scalar.activation(
                            aggs[:, :], aggs_ps[b],
                            mybir.ActivationFunctionType.Copy,
                        )
                        if L < 2:
                            hp = ps_dense.tile([F, P], f32, tag="hp")
                            nc.tensor.matmul(
                                hp[:, :], lhsT=w_sb[L][:, :], rhs=aggs[:, :],
                                start=True, stop=True,
                            )
                            hT = apool.tile([F, P], f32, tag="hT")
                            nc.scalar.activation(
                                hT[:, :], hp[:, :],
                                mybir.ActivationFunctionType.Relu,
                                bias=b_sb[L][:, :],
                            )
                            tp = ps_tr.tile([P, F], f32, tag="tp")
                            nc.tensor.matmul(
                                tp[:, :], lhsT=hT[:, :], rhs=ident_sb[:, :],
                                is_transpose=True, start=True, stop=True,
                            )
                            hout = hpool.tile([P, F], f32, tag="hout")
                            nc.scalar.activation(
                                hout[:, :], tp[:, :],
                                mybir.ActivationFunctionType.Copy,
                            )
                            # hsc for the next layer: hout * self_coef (fp16)
                            nc.vector.tensor_scalar(
                                out=hsc_sb[L][:, b * F:(b + 1) * F],
                                in0=hout[:, :],
                                scalar1=scp_sb[:, b:b + 1],
                                scalar2=None,
                                op0=mybir.AluOpType.mult,
                            )
                            nc.sync.dma_start(
                                out=hloc[L][b * P: (b + 1) * P, :], in_=hout[:, :]
                            )
                        else:
                            yp = ps_dense.tile([1, P], f32, tag="hp", name="yp")
                            nc.tensor.matmul(
                                yp[:, :], lhsT=w_sb[2][:, :], rhs=aggs[:, :],
                                start=True, stop=True,
                            )
                            nc.scalar.activation(
                                y_sb[:, b * P: (b + 1) * P], yp[:, :],
                                mybir.ActivationFunctionType.Copy,
                            )
                if L < 2 and not skip_collective:
                    nc.gpsimd.collective_compute(
                        "AllGather",
                        mybir.AluOpType.bypass,
                        replica_groups=[list(range(C))],
                        ins=[hloc[L].ap().opt()],
                        outs=[htab[L].ap().opt()],
                    )
            nc.sync.dma_start(out=y_d[:, :], in_=y_sb[:, :])

    nc.compile()
    return nc


def make_in_maps(meta, W0, b0, W1, b1, W2):
    C = meta["C"]; F = meta["F"]; K = meta["K"]
    CHMAX, SMAX = eq_widths(meta)
    # iotaT[p, (s d t)] = s*128 + d expanded along t: s=0 block CHMAX wide,
    # s>0 blocks SMAX wide (matches build_nc slicing)
    segs = []
    for s in range(meta["bpc"]):
        vals = (np.arange(P, dtype=np.float16) + s * P).reshape(P, 1)
        segs.append(np.tile(vals, (1, SMAX)).reshape(-1))
    iotaT = np.tile(np.concatenate(segs)[None, :], (P, 1)).astype(np.float16)
    id128 = np.eye(P, dtype=np.float16)
    common = dict(
        xt=meta["xt"],
        iotaT=iotaT,
        id128=id128,
        ident=np.eye(F, dtype=np.float32),
        w0=np.asarray(W0, np.float32),
        w1=np.asarray(W1, np.float32),
        w2=np.asarray(W2, np.float32).reshape(F, 1),
        b0=np.asarray(b0, np.float32).reshape(F, 1),
        b1=np.asarray(b1, np.float32).reshape(F, 1),
    )
    in_maps = []
    for c in range(C):
        im = dict(common)
        im["coef"] = meta["coef"][c]
        im["dstl"] = meta["dstl"][c]
        im["xsc"] = meta["xsc"][c]
        im["scp"] = meta["scp"][c]
        im["ncnt"] = meta["ncnt"][c].reshape(1, -1)
        for k in range(K):
            im[f"gix{k}"] = meta["gidx16"][k][c]
        in_maps.append(im)
    return in_maps


def assemble_output(meta, results, b2):
    C = meta["C"]
    rows_pc = meta["rows_pc"]
    ys = np.stack([np.asarray(results[c]["y"]).reshape(rows_pc) for c in range(C)])
    y = ys[meta["core_of"], meta["pos_of"]] + np.float32(np.asarray(b2).reshape(-1)[0])
    return y.astype(np.float32)


def kernel(x, edge_src, edge_dst, edge_weights, W0, b0, W1, b1, W2, b2,
           n_queues=4, trace=False):
    """Harness entry point: full inputs in, full [N] float32 output."""
    x = np.asarray(x)
    meta = preprocess(x, np.asarray(edge_src), np.asarray(edge_dst),
                      np.asarray(edge_weights))
    nc = build_nc(meta, n_queues=n_queues)
    in_maps = make_in_maps(meta, W0, b0, W1, b1, W2)
    last_err = None
    for attempt in range(3):
        try:
            res = bass_utils.run_bass_kernel_spmd(
                nc, in_maps, core_ids=list(range(meta["C"])), trace=trace
            )
            y = assemble_output(meta, res.results, b2)
            kernel.last_result = res
            kernel.last_nc = nc
            kernel.last_meta = meta
            kernel.last_in_maps = in_maps
            return y
        except Exception as e:  # transient accelerator failures: retry
            last_err = e
    raise last_err

